# revision 19
# baseline (speedup 1.0000x reference)
"""Linear attention (elu(x)+1 feature map) Bass/Tile kernel for Trainium2.

Problem: B=4, H=16, S=4096, D=64, fp32.
  Qf = elu(Q)+1; Kf = elu(K)+1
  KV = Kf^T (V*mask);  Ksum = Kf^T mask
  out = (Qf @ KV) / (Qf . Ksum)

Sharding: 64 (b,h) pairs data-parallel over 8 cores, 8 pairs/core, no
collectives. Pairs processed in 4 groups of 2 (A/B) so the phase-B
matmuls use full 128 partitions.

v5 design (timeline: v1 384us -> v2 346 -> v3 117 -> v4 95):
- Host repacks inputs: all of Q/K/V cast to bf16 (validated: fro err
  stays 2.9e-3, far under the 2e-2 gate, because the on-chip pipeline
  is bf16 anyway) and pair-interleaved to [group, s, pair, d]; V gains
  a 65th column carrying the mask (mask folded into V only when not
  all-ones - exact for any mask). Upshot: every DMA is one fully
  contiguous 2KiB-run transfer, Q and V load with ONE call per group,
  K one per half-group - issue overhead (SWDGE/HWDGE) drops ~4x, HBM
  traffic is 17MB/core instead of 34.
- s = blk*1024 + p*8 + j interleaving: KV accumulation is order-free,
  and phase-B chunk j covers s = 8p+j so the output tile drains as
  2KiB-contiguous rows too.
- All matmuls bf16 incl. the Q transposes; ACT exp/relu read the
  transpose PSUM directly; elu+1 = min(exp,1)+relu assembled by
  tensor_scalar_min + tensor_tensor add on DVE (scalar_tensor_tensor
  and GPSIMD bulk ops measured 4-10x slower than modeled - avoided).
  relu(K) runs on DVE to balance ACT vs DVE.
- Both pairs' [KV|Ksum] accumulate in ONE PSUM bank at partition
  offsets 0/64. Separate den matmul keeps the out matmuls exactly
  bank-aligned ([128,8,128] over two banks); normalization is one
  512-col DVE op per (block, pair); output stored bf16, upcast on host.
- Q/V DMAs issue from the GPSIMD SWDGE queue, K/out from the SP HWDGE
  queue, spreading issue cost off the critical sequencer.
"""

import numpy as np

import concourse.bass as bass
import concourse.mybir as mybir
import concourse.tile as tile
from concourse.bass_utils import run_bass_kernel_spmd
from concourse.masks import make_identity

F32 = mybir.dt.float32
BF16 = mybir.dt.bfloat16
AF = mybir.ActivationFunctionType
ALU = mybir.AluOpType

N_CORES = 8
PAIRS = 8          # (b,h) pairs per core
S = 4096
D = 64
E = D + 1          # V is host-padded with the mask column
NB = 4             # blocks (of 1024 rows) per pair
NJ = 8             # s-rows per partition per block (s = blk*1024 + p*8 + j)
NG = PAIRS // 2    # pair-groups
NH = 2             # half-groups (2 blocks each) per group


def build_bass() -> bass.Bass:
    from contextlib import ExitStack
    from concourse.bacc import Bacc
    nc = Bacc()
    # pair-interleaved host layouts
    Qh = nc.dram_tensor("Q", [NG, S, 2, D], BF16, kind="ExternalInput")
    Kh = nc.dram_tensor("K", [NG, S, 2, D], BF16, kind="ExternalInput")
    Vh = nc.dram_tensor("V", [NG, S, 2, E], BF16, kind="ExternalInput")
    Oh = nc.dram_tensor("O", [PAIRS, S, D], BF16, kind="ExternalOutput")

    # s = blk*1024 + p*8 + j
    Qv = [Qh[g].rearrange("(b p j) u d -> p b j u d", b=NB, p=128, j=NJ)
          for g in range(NG)]
    Kv = [Kh[g].rearrange("(h c p j) u d -> h p c j u d",
                          h=NH, c=2, p=128, j=NJ) for g in range(NG)]
    Vv = [Vh[g].rearrange("(b p j) u e -> p b j u e", b=NB, p=128, j=NJ)
          for g in range(NG)]
    Op = [Oh[p].rearrange("(b p j) d -> p b j d", b=NB, p=128, j=NJ)
          for p in range(PAIRS)]

    with tile.TileContext(nc) as tc, ExitStack() as ctx, \
            nc.allow_low_precision("bf16 pipeline; fro gate is 2e-2"):
        consts = ctx.enter_context(tc.tile_pool(name="consts", bufs=1))
        qr_pool = ctx.enter_context(tc.tile_pool(name="qr", bufs=2))
        kr_pool = ctx.enter_context(tc.tile_pool(name="kr", bufs=3))
        vm_pool = ctx.enter_context(tc.tile_pool(name="vm", bufs=2))
        exk_pool = ctx.enter_context(tc.tile_pool(name="exk", bufs=2))
        rlk_pool = ctx.enter_context(tc.tile_pool(name="rlk", bufs=2))
        mnk_pool = ctx.enter_context(tc.tile_pool(name="mnk", bufs=2))
        kf_pool = ctx.enter_context(tc.tile_pool(name="kf", bufs=2))
        exq_pool = ctx.enter_context(tc.tile_pool(name="exq", bufs=2))
        rlq_pool = ctx.enter_context(tc.tile_pool(name="rlq", bufs=2))
        mnq_pool = ctx.enter_context(tc.tile_pool(name="mnq", bufs=2))
        qtf_pool = ctx.enter_context(tc.tile_pool(name="qtf", bufs=2))
        bd_pool = ctx.enter_context(tc.tile_pool(name="bd", bufs=2))
        ks_pool = ctx.enter_context(tc.tile_pool(name="ks", bufs=2))
        rec_pool = ctx.enter_context(tc.tile_pool(name="rec", bufs=3))
        osb_pool = ctx.enter_context(tc.tile_pool(name="osb", bufs=2))
        tp_psum = ctx.enter_context(tc.tile_pool(name="tpps", bufs=2, space="PSUM"))
        kv_psum = ctx.enter_context(tc.tile_pool(name="kvps", bufs=1, space="PSUM"))
        ob_psum = ctx.enter_context(tc.tile_pool(name="obps", bufs=2, space="PSUM"))
        dn_psum = ctx.enter_context(tc.tile_pool(name="dnps", bufs=1, space="PSUM"))

        identity = consts.tile([128, 128], BF16)
        make_identity(nc, identity)

        kv_ps_g = [None] * NG
        qtf_g = [None] * NG
        bd_g = [None] * NG
        ks2_g = [None] * NG

        def phase_a(g):
            kv_ps = kv_psum.tile([128, 2 * E], F32, tag="kv", name=f"kv_{g}")
            kv_ps_g[g] = kv_ps
            qtf = qtf_pool.tile([128, NB, NJ, 128], BF16, tag="qtf",
                                name=f"qtf_{g}")
            qtf_g[g] = qtf
            # group Q and V tiles, loaded per half-group (faster ramp)
            qraw = qr_pool.tile([128, NB, NJ, 2, D], BF16, tag="qr",
                                name=f"qr_{g}")
            vm = vm_pool.tile([128, NB, NJ, 2, E], BF16, tag="vm",
                              name=f"vm_{g}")
            for hh in range(NH):
                b0 = 2 * hh
                nc.gpsimd.dma_start(out=qraw[:, b0:b0 + 2],
                                    in_=Qv[g][:, b0:b0 + 2])
                nc.gpsimd.dma_start(out=vm[:, b0:b0 + 2],
                                    in_=Vv[g][:, b0:b0 + 2])

            for h in range(NH):
                kraw = kr_pool.tile([128, 2, NJ, 2, D], BF16, tag="kr",
                                    name=f"kr_{g}_{h}")
                nc.sync.dma_start(out=kraw, in_=Kv[g][h])

                # kf = min(exp(K),1) + relu(K)  (bf16, both pairs fused)
                exk = exk_pool.tile([128, 2, NJ, 2, D], BF16, tag="exk",
                                    name=f"exk_{g}_{h}")
                rlk = rlk_pool.tile([128, 2, NJ, 2, D], BF16, tag="rlk",
                                    name=f"rlk_{g}_{h}")
                mnk = mnk_pool.tile([128, 2, NJ, 2, D], BF16, tag="mnk",
                                    name=f"mnk_{g}_{h}")
                kf = kf_pool.tile([128, 2, NJ, 2, D], BF16, tag="kf",
                                  name=f"kf_{g}_{h}")
                nc.scalar.activation(exk, kraw, AF.Exp)
                nc.vector.tensor_scalar_max(rlk, kraw, 0.0)
                nc.vector.tensor_scalar_min(mnk, exk, 1.0)
                nc.vector.tensor_tensor(out=kf, in0=mnk, in1=rlk, op=ALU.add)

                for c in range(2):
                    blk = 2 * h + c
                    # Q: PE-transpose bf16; exp/relu read PSUM directly
                    exq = exq_pool.tile([128, NJ, 128], BF16, tag="exq",
                                        name=f"exq_{g}_{blk}")
                    rlq = rlq_pool.tile([128, NJ, 128], BF16, tag="rlq",
                                        name=f"rlq_{g}_{blk}")
                    mnq = mnq_pool.tile([128, NJ, 128], BF16, tag="mnq",
                                        name=f"mnq_{g}_{blk}")
                    for half in range(2):
                        tp = tp_psum.tile([128, 4, 128], BF16, tag="tp",
                                          name=f"tp_{g}_{blk}_{half}")
                        for jj in range(4):
                            j = half * 4 + jj
                            nc.tensor.transpose(tp[:, jj, :],
                                                qraw[:, blk, j], identity)
                        sl = slice(half * 4, half * 4 + 4)
                        nc.scalar.activation(exq[:, sl, :], tp, AF.Exp)
                        nc.scalar.activation(rlq[:, sl, :], tp, AF.Relu)
                    nc.vector.tensor_scalar_min(mnq, exq, 1.0)
                    nc.vector.tensor_tensor(out=qtf[:, blk], in0=mnq, in1=rlq,
                                            op=ALU.add)

                    # KV accumulation, pairs fused: out [128,(u,e)]
                    # rows (u,d); off-diagonal blocks are junk
                    for j in range(NJ):
                        cc = blk * NJ + j
                        nc.tensor.matmul(
                            kv_ps, lhsT=kf[:, c, j], rhs=vm[:, blk, j],
                            start=(cc == 0), stop=(cc == NB * NJ - 1))

        def extract_bd(g):
            kv_ps = kv_ps_g[g]
            bd = bd_pool.tile([128, 128], BF16, tag="bd", name=f"bd_{g}")
            ks2 = ks_pool.tile([128, 2], BF16, tag="ks2", name=f"ks2_{g}")
            nc.vector.memset(bd, 0.0)
            nc.vector.memset(ks2, 0.0)
            nc.vector.tensor_copy(out=bd[0:64, 0:64], in_=kv_ps[0:64, 0:D])
            nc.vector.tensor_copy(out=bd[64:128, 64:128],
                                  in_=kv_ps[64:128, E:E + D])
            nc.vector.tensor_copy(out=ks2[0:64, 0:1], in_=kv_ps[0:64, D:E])
            nc.vector.tensor_copy(out=ks2[64:128, 1:2],
                                  in_=kv_ps[64:128, E + D:2 * E])
            bd_g[g], ks2_g[g] = bd, ks2

        def phase_b(g):
            bd, ks2 = bd_g[g], ks2_g[g]
            qtf = qtf_g[g]
            osb = [osb_pool.tile([128, NB, NJ, D], BF16, tag=f"osb{u}",
                                 name=f"osb_{g}_{u}") for u in range(2)]
            for blk in range(NB):
                if blk == 2:
                    for u in range(2):
                        nc.sync.dma_start(out=Op[2 * g + u][:, 0:2],
                                          in_=osb[u][:, 0:2])
                den_ps = dn_psum.tile([128, NJ, 2], F32, tag="dn",
                                      name=f"dn_{g}_{blk}")
                # 8 chunk matmuls fill one 2-bank PSUM tile
                ob = ob_psum.tile([128, NJ, 128], F32, tag="ob",
                                  name=f"ob_{g}_{blk}")
                for j in range(NJ):
                    lhsT = qtf[:, blk, j, :]
                    nc.tensor.matmul(ob[:, j, :], lhsT=lhsT, rhs=bd,
                                     start=True, stop=True,
                                     skip_group_check=True)
                    nc.tensor.matmul(den_ps[:, j, :], lhsT=lhsT,
                                     rhs=ks2, start=True, stop=True,
                                     skip_group_check=True)
                rec = rec_pool.tile([128, 2, NJ], BF16, tag="rec",
                                    name=f"rec_{g}_{blk}")
                nc.vector.reciprocal(rec.rearrange("p u j -> p j u"), den_ps)
                for u in range(2):
                    nc.vector.tensor_tensor(
                        out=osb[u][:, blk],
                        in0=ob[:, :, u * D:(u + 1) * D],
                        in1=rec[:, u].to_broadcast([128, NJ, D]),
                        op=ALU.mult)
            for u in range(2):
                nc.sync.dma_start(out=Op[2 * g + u][:, 2:4],
                                  in_=osb[u][:, 2:4])

        # staggered emission: A(0) A(1) B(0) A(2) B(1) A(3) B(2) B(3)
        phase_a(0)
        extract_bd(0)
        for g in range(1, NG):
            phase_a(g)
            extract_bd(g)
            phase_b(g - 1)
        phase_b(NG - 1)

    nc.finalize()
    return nc


_NC_CACHE = None


def _get_nc():
    global _NC_CACHE
    if _NC_CACHE is None:
        _NC_CACHE = build_bass()
    return _NC_CACHE


def kernel(Q: np.ndarray, K: np.ndarray, V: np.ndarray, mask: np.ndarray,
           _trace: bool = False):
    import ml_dtypes
    BF = ml_dtypes.bfloat16
    B, H = 4, 16
    NP = B * H
    per = NP // N_CORES
    ng_total = NP // 2
    # pair-interleaved bf16 host layouts: [group, s, pair, d]
    Qi = np.ascontiguousarray(
        np.asarray(Q, dtype=np.float32).reshape(ng_total, 2, S, D)
        .transpose(0, 2, 1, 3).astype(BF))
    Ki = np.ascontiguousarray(
        np.asarray(K, dtype=np.float32).reshape(ng_total, 2, S, D)
        .transpose(0, 2, 1, 3).astype(BF))
    Vr = np.asarray(V, dtype=np.float32).reshape(NP, S, D)
    Mr = np.asarray(mask, dtype=np.float32).reshape(NP, S)
    # V packed with the mask column: exact for any mask, free when ones
    Vpk = np.empty((NP, S, E), dtype=BF)
    if np.all(Mr == 1.0):
        Vpk[:, :, 0:D] = Vr
    else:
        Vpk[:, :, 0:D] = Vr * Mr[:, :, None]
    Vpk[:, :, D] = Mr
    Vi = np.ascontiguousarray(
        Vpk.reshape(ng_total, 2, S, E).transpose(0, 2, 1, 3))

    in_maps = []
    gper = per // 2
    for i in range(N_CORES):
        sl = slice(i * gper, (i + 1) * gper)
        in_maps.append({
            "Q": np.ascontiguousarray(Qi[sl]),
            "K": np.ascontiguousarray(Ki[sl]),
            "V": np.ascontiguousarray(Vi[sl]),
        })

    nc = _get_nc()
    res = run_bass_kernel_spmd(nc, in_maps, core_ids=list(range(N_CORES)),
                               trace=_trace)
    out = np.concatenate(
        [np.asarray(r["O"]).astype(np.float32) for r in res.results], axis=0)
    if _trace:
        kernel._last_results = res
    return out.reshape(B, H, S, D)


# revision 20
# speedup vs baseline: 1.1472x; 1.1472x over previous
"""Linear attention (elu(x)+1 feature map) Bass/Tile kernel for Trainium2.

Problem: B=4, H=16, S=4096, D=64, fp32.
  Qf = elu(Q)+1; Kf = elu(K)+1
  KV = Kf^T (V*mask);  Ksum = Kf^T mask
  out = (Qf @ KV) / (Qf . Ksum)

Sharding: 64 (b,h) pairs data-parallel over 8 cores, 8 pairs/core, no
collectives. Pairs processed in 4 groups of 2 (A/B) so the phase-B
matmuls use full 128 partitions.

v5 design (timeline: v1 384us -> v2 346 -> v3 117 -> v4 95):
- Host repacks inputs: all of Q/K/V cast to bf16 (validated: fro err
  stays 2.9e-3, far under the 2e-2 gate, because the on-chip pipeline
  is bf16 anyway) and pair-interleaved to [group, s, pair, d]; V gains
  a 65th column carrying the mask (mask folded into V only when not
  all-ones - exact for any mask). Upshot: every DMA is one fully
  contiguous 2KiB-run transfer, Q and V load with ONE call per group,
  K one per half-group - issue overhead (SWDGE/HWDGE) drops ~4x, HBM
  traffic is 17MB/core instead of 34.
- s = blk*1024 + p*8 + j interleaving: KV accumulation is order-free,
  and phase-B chunk j covers s = 8p+j so the output tile drains as
  2KiB-contiguous rows too.
- All matmuls bf16 incl. the Q transposes; ACT exp/relu read the
  transpose PSUM directly; elu+1 = min(exp,1)+relu assembled by
  tensor_scalar_min + tensor_tensor add on DVE (scalar_tensor_tensor
  and GPSIMD bulk ops measured 4-10x slower than modeled - avoided).
  relu(K) runs on DVE to balance ACT vs DVE.
- Both pairs' [KV|Ksum] accumulate in ONE PSUM bank at partition
  offsets 0/64. Separate den matmul keeps the out matmuls exactly
  bank-aligned ([128,8,128] over two banks); normalization is one
  512-col DVE op per (block, pair); output stored bf16, upcast on host.
- Q/V DMAs issue from the GPSIMD SWDGE queue, K/out from the SP HWDGE
  queue, spreading issue cost off the critical sequencer.
"""

import numpy as np

import concourse.bass as bass
import concourse.mybir as mybir
import concourse.tile as tile
from concourse.bass_utils import run_bass_kernel_spmd
from concourse.masks import make_identity

F32 = mybir.dt.float32
BF16 = mybir.dt.bfloat16
AF = mybir.ActivationFunctionType
ALU = mybir.AluOpType

N_CORES = 8
PAIRS = 8          # (b,h) pairs per core
S = 4096
D = 64
E = D + 1          # V is host-padded with the mask column
NB = 4             # blocks (of 1024 rows) per pair
NJ = 8             # s-rows per partition per block (s = blk*1024 + p*8 + j)
NG = PAIRS // 2    # pair-groups
NH = 2             # half-groups (2 blocks each) per group


def build_bass() -> bass.Bass:
    from contextlib import ExitStack
    from concourse.bacc import Bacc
    nc = Bacc()
    # pair-interleaved host layouts
    Qh = nc.dram_tensor("Q", [NG, S, 2, D], BF16, kind="ExternalInput")
    Kh = nc.dram_tensor("K", [NG, S, 2, D], BF16, kind="ExternalInput")
    Vh = nc.dram_tensor("V", [NG, S, 2, E], BF16, kind="ExternalInput")
    Oh = nc.dram_tensor("O", [PAIRS, S, D], BF16, kind="ExternalOutput")

    # s = blk*1024 + p*8 + j
    Qv = [Qh[g].rearrange("(b p j) u d -> p b j u d", b=NB, p=128, j=NJ)
          for g in range(NG)]
    Kv = [Kh[g].rearrange("(h c p j) u d -> h p c j u d",
                          h=NH, c=2, p=128, j=NJ) for g in range(NG)]
    Vv = [Vh[g].rearrange("(b p j) u e -> p b j u e", b=NB, p=128, j=NJ)
          for g in range(NG)]
    Op = [Oh[p].rearrange("(b p j) d -> p b j d", b=NB, p=128, j=NJ)
          for p in range(PAIRS)]

    with tile.TileContext(nc) as tc, ExitStack() as ctx, \
            nc.allow_low_precision("bf16 pipeline; fro gate is 2e-2"):
        consts = ctx.enter_context(tc.tile_pool(name="consts", bufs=1))
        qr_pool = ctx.enter_context(tc.tile_pool(name="qr", bufs=2))
        kr_pool = ctx.enter_context(tc.tile_pool(name="kr", bufs=3))
        vm_pool = ctx.enter_context(tc.tile_pool(name="vm", bufs=2))
        exk_pool = ctx.enter_context(tc.tile_pool(name="exk", bufs=2))
        rlk_pool = ctx.enter_context(tc.tile_pool(name="rlk", bufs=2))
        mnk_pool = ctx.enter_context(tc.tile_pool(name="mnk", bufs=2))
        kf_pool = ctx.enter_context(tc.tile_pool(name="kf", bufs=2))
        exq_pool = ctx.enter_context(tc.tile_pool(name="exq", bufs=2))
        rlq_pool = ctx.enter_context(tc.tile_pool(name="rlq", bufs=2))
        mnq_pool = ctx.enter_context(tc.tile_pool(name="mnq", bufs=2))
        qtf_pool = ctx.enter_context(tc.tile_pool(name="qtf", bufs=2))
        bd_pool = ctx.enter_context(tc.tile_pool(name="bd", bufs=2))
        ks_pool = ctx.enter_context(tc.tile_pool(name="ks", bufs=2))
        rec_pool = ctx.enter_context(tc.tile_pool(name="rec", bufs=3))
        osb_pool = ctx.enter_context(tc.tile_pool(name="osb", bufs=2))
        tp_psum = ctx.enter_context(tc.tile_pool(name="tpps", bufs=2, space="PSUM"))
        kv_psum = ctx.enter_context(tc.tile_pool(name="kvps", bufs=1, space="PSUM"))
        ob_psum = ctx.enter_context(tc.tile_pool(name="obps", bufs=2, space="PSUM"))
        dn_psum = ctx.enter_context(tc.tile_pool(name="dnps", bufs=1, space="PSUM"))

        identity = consts.tile([128, 128], BF16)
        make_identity(nc, identity)

        kv_ps_g = [None] * NG
        qtf_g = [None] * NG
        bd_g = [None] * NG
        ks2_g = [None] * NG

        def phase_a(g):
            kv_ps = kv_psum.tile([128, E], F32, tag="kv", name=f"kv_{g}")
            kv_ps_g[g] = kv_ps
            qtf = qtf_pool.tile([128, NB, NJ, 128], BF16, tag="qtf",
                                name=f"qtf_{g}")
            qtf_g[g] = qtf
            # group Q and V tiles, loaded per half-group (faster ramp)
            qraw = qr_pool.tile([128, NB, NJ, 2, D], BF16, tag="qr",
                                name=f"qr_{g}")
            vm = vm_pool.tile([128, NB, NJ, 2, E], BF16, tag="vm",
                              name=f"vm_{g}")
            for hh in range(NH):
                b0 = 2 * hh
                nc.gpsimd.dma_start(out=qraw[:, b0:b0 + 2],
                                    in_=Qv[g][:, b0:b0 + 2])
                nc.gpsimd.dma_start(out=vm[:, b0:b0 + 2],
                                    in_=Vv[g][:, b0:b0 + 2])

            for h in range(NH):
                kraw = kr_pool.tile([128, 2, NJ, 2, D], BF16, tag="kr",
                                    name=f"kr_{g}_{h}")
                nc.sync.dma_start(out=kraw, in_=Kv[g][h])

                # kf = min(exp(K),1) + relu(K)  (bf16, both pairs fused)
                exk = exk_pool.tile([128, 2, NJ, 2, D], BF16, tag="exk",
                                    name=f"exk_{g}_{h}")
                rlk = rlk_pool.tile([128, 2, NJ, 2, D], BF16, tag="rlk",
                                    name=f"rlk_{g}_{h}")
                mnk = mnk_pool.tile([128, 2, NJ, 2, D], BF16, tag="mnk",
                                    name=f"mnk_{g}_{h}")
                kf = kf_pool.tile([128, 2, NJ, 2, D], BF16, tag="kf",
                                  name=f"kf_{g}_{h}")
                nc.scalar.activation(exk, kraw, AF.Exp)
                nc.vector.tensor_scalar_max(rlk, kraw, 0.0)
                nc.vector.tensor_scalar_min(mnk, exk, 1.0)
                nc.vector.tensor_tensor(out=kf, in0=mnk, in1=rlk, op=ALU.add)

                for c in range(2):
                    blk = 2 * h + c
                    # Q: PE-transpose bf16; exp/relu read PSUM directly
                    exq = exq_pool.tile([128, NJ, 128], BF16, tag="exq",
                                        name=f"exq_{g}_{blk}")
                    rlq = rlq_pool.tile([128, NJ, 128], BF16, tag="rlq",
                                        name=f"rlq_{g}_{blk}")
                    mnq = mnq_pool.tile([128, NJ, 128], BF16, tag="mnq",
                                        name=f"mnq_{g}_{blk}")
                    for half in range(2):
                        tp = tp_psum.tile([128, 4, 128], BF16, tag="tp",
                                          name=f"tp_{g}_{blk}_{half}")
                        for jj in range(4):
                            j = half * 4 + jj
                            nc.tensor.transpose(tp[:, jj, :],
                                                qraw[:, blk, j], identity)
                        sl = slice(half * 4, half * 4 + 4)
                        nc.scalar.activation(exq[:, sl, :], tp, AF.Exp)
                        nc.scalar.activation(rlq[:, sl, :], tp, AF.Relu)
                    nc.vector.tensor_scalar_min(mnq, exq, 1.0)
                    nc.vector.tensor_tensor(out=qtf[:, blk], in0=mnq, in1=rlq,
                                            op=ALU.add)

                    # KV accumulation: pair u in PSUM partitions u*64..
                    for j in range(NJ):
                        cc = blk * NJ + j
                        for u in range(2):
                            nc.tensor.matmul(
                                kv_ps[u * 64:(u + 1) * 64, :],
                                lhsT=kf[:, c, j, u], rhs=vm[:, blk, j, u],
                                start=(cc == 0), stop=(cc == NB * NJ - 1),
                                skip_group_check=True)

        def extract_bd(g):
            kv_ps = kv_ps_g[g]
            bd = bd_pool.tile([128, 128], BF16, tag="bd", name=f"bd_{g}")
            ks2 = ks_pool.tile([128, 2], BF16, tag="ks2", name=f"ks2_{g}")
            nc.vector.memset(bd, 0.0)
            nc.vector.memset(ks2, 0.0)
            nc.vector.tensor_copy(out=bd[0:64, 0:64], in_=kv_ps[0:64, 0:D])
            nc.vector.tensor_copy(out=bd[64:128, 64:128], in_=kv_ps[64:128, 0:D])
            nc.vector.tensor_copy(out=ks2[0:64, 0:1], in_=kv_ps[0:64, D:E])
            nc.vector.tensor_copy(out=ks2[64:128, 1:2], in_=kv_ps[64:128, D:E])
            bd_g[g], ks2_g[g] = bd, ks2

        def phase_b(g):
            bd, ks2 = bd_g[g], ks2_g[g]
            qtf = qtf_g[g]
            osb = [osb_pool.tile([128, NB, NJ, D], BF16, tag=f"osb{u}",
                                 name=f"osb_{g}_{u}") for u in range(2)]
            for blk in range(NB):
                if blk == 2:
                    for u in range(2):
                        nc.sync.dma_start(out=Op[2 * g + u][:, 0:2],
                                          in_=osb[u][:, 0:2])
                den_ps = dn_psum.tile([128, NJ, 2], F32, tag="dn",
                                      name=f"dn_{g}_{blk}")
                # 8 chunk matmuls fill one 2-bank PSUM tile
                ob = ob_psum.tile([128, NJ, 128], F32, tag="ob",
                                  name=f"ob_{g}_{blk}")
                for j in range(NJ):
                    lhsT = qtf[:, blk, j, :]
                    nc.tensor.matmul(ob[:, j, :], lhsT=lhsT, rhs=bd,
                                     start=True, stop=True,
                                     skip_group_check=True)
                    nc.tensor.matmul(den_ps[:, j, :], lhsT=lhsT,
                                     rhs=ks2, start=True, stop=True,
                                     skip_group_check=True)
                rec = rec_pool.tile([128, 2, NJ], BF16, tag="rec",
                                    name=f"rec_{g}_{blk}")
                nc.vector.reciprocal(rec.rearrange("p u j -> p j u"), den_ps)
                for u in range(2):
                    nc.vector.tensor_tensor(
                        out=osb[u][:, blk],
                        in0=ob[:, :, u * D:(u + 1) * D],
                        in1=rec[:, u].to_broadcast([128, NJ, D]),
                        op=ALU.mult)
            for u in range(2):
                nc.sync.dma_start(out=Op[2 * g + u][:, 2:4],
                                  in_=osb[u][:, 2:4])

        # staggered emission: A(0) A(1) B(0) A(2) B(1) A(3) B(2) B(3)
        phase_a(0)
        extract_bd(0)
        for g in range(1, NG):
            phase_a(g)
            extract_bd(g)
            phase_b(g - 1)
        phase_b(NG - 1)

    nc.finalize()
    return nc


_NC_CACHE = None


def _get_nc():
    global _NC_CACHE
    if _NC_CACHE is None:
        _NC_CACHE = build_bass()
    return _NC_CACHE


def kernel(Q: np.ndarray, K: np.ndarray, V: np.ndarray, mask: np.ndarray,
           _trace: bool = False):
    import ml_dtypes
    BF = ml_dtypes.bfloat16
    B, H = 4, 16
    NP = B * H
    per = NP // N_CORES
    ng_total = NP // 2
    # pair-interleaved bf16 host layouts: [group, s, pair, d]
    Qi = np.ascontiguousarray(
        np.asarray(Q, dtype=np.float32).reshape(ng_total, 2, S, D)
        .transpose(0, 2, 1, 3).astype(BF))
    Ki = np.ascontiguousarray(
        np.asarray(K, dtype=np.float32).reshape(ng_total, 2, S, D)
        .transpose(0, 2, 1, 3).astype(BF))
    Vr = np.asarray(V, dtype=np.float32).reshape(NP, S, D)
    Mr = np.asarray(mask, dtype=np.float32).reshape(NP, S)
    # V packed with the mask column: exact for any mask, free when ones
    Vpk = np.empty((NP, S, E), dtype=BF)
    if np.all(Mr == 1.0):
        Vpk[:, :, 0:D] = Vr
    else:
        Vpk[:, :, 0:D] = Vr * Mr[:, :, None]
    Vpk[:, :, D] = Mr
    Vi = np.ascontiguousarray(
        Vpk.reshape(ng_total, 2, S, E).transpose(0, 2, 1, 3))

    in_maps = []
    gper = per // 2
    for i in range(N_CORES):
        sl = slice(i * gper, (i + 1) * gper)
        in_maps.append({
            "Q": np.ascontiguousarray(Qi[sl]),
            "K": np.ascontiguousarray(Ki[sl]),
            "V": np.ascontiguousarray(Vi[sl]),
        })

    nc = _get_nc()
    res = run_bass_kernel_spmd(nc, in_maps, core_ids=list(range(N_CORES)),
                               trace=_trace)
    out = np.concatenate(
        [np.asarray(r["O"]).astype(np.float32) for r in res.results], axis=0)
    if _trace:
        kernel._last_results = res
    return out.reshape(B, H, S, D)
